# revision 2
# baseline (speedup 1.0000x reference)
"""Expert-parallel MoE grouped-MLP kernel for 8 TRN2 NeuronCores.

Computes, for tokens t in expert e's contiguous row range (rows of x are
sorted by expert; boundaries come from num_tokens_per_expert):

    out[t] = bf16( relu(bf16(x[t]) @ w_up[e].T)^2 @ w_down[e].T )  -> f32

Strategy (expert parallelism): core e owns expert e. The host does the
dispatch - slices x by expert boundaries, transposes/retiles to DMA-friendly
layouts, casts to bf16 - so each core runs two dense back-to-back bf16
matmul chains entirely on-chip with zero routing logic:

    hT[hh, t] = sum_d w_upT[d, hh] * xT[d, t]        (mm1, PSUM f32)
    hT       <- relu(hT)^2  (cast bf16)               (DVE, fused op)
    oT[dd, t] = sum_hh w_downT[hh, dd] * hT[hh, t]    (mm2, PSUM f32)

Ramp/latency design (the steady-state PE stream is already at the N=512
issue bound of ~213.5ns/MM, so the wins are at the edges):
  - 8 warmup matmuls on a zeroed tile at kernel start keep the PE busy
    during the input-DMA ramp so the HAM clock gate opens (1.2->2.4GHz)
    before the first real matmul.
  - Input DMAs are split over BOTH HWDGE queues (sync row: x, scalar row:
    weights) so each stream gets its own share of HBM bandwidth, and are
    emitted in consumption order with a fine-grained head: x arrives in
    256-token halves, w_up's first 512-col block in four 128-col segments,
    so mm1's first PSUM group completes as early as possible and the PE
    never starves afterwards.
  - Host pre-tiles every DRAM operand so each DMA's per-partition lines
    are contiguous (>=512B fragments, most >=2KB) for near-peak HBM rate.
  - mm1 runs N=256 groups (matching the x half-tiles; measured MM spacing
    at N=256 still hides LDWEIGHTS), mm2 runs N=512 groups with the last
    group split 2x256 so the final copy+output-DMA overlaps matmuls.
"""

import os

import numpy as np
import ml_dtypes

N_CORES = 8
BF16 = ml_dtypes.bfloat16
P = 128          # SBUF/PSUM partitions
TT = 512         # token tile (mm2 free dim / one PSUM bank of f32)
HT = 256         # token half-tile (mm1 free dim / x DMA granularity)

_cache = {}
_wcache = {}  # weight digest -> (host retiled copies, device arrays)
LAST_RESULT = None  # BassKernelResults of the most recent run (for profiling)


def _build(D, H, cap, repeat=1, ablate=()):
    """Compile the per-core Bass program for fixed token capacity `cap`.

    repeat>1 emits the whole body N times into one NEFF (tags shared, so
    iterations serialize through tile reuse) - used only by the timing
    harness to measure per-iteration device time differentially.
    """
    import concourse.mybir as mybir
    import concourse.tile as tile
    from concourse import bacc

    f32 = mybir.dt.float32
    bf16 = mybir.dt.bfloat16

    nc = bacc.Bacc("TRN2", target_bir_lowering=False, debug=False,
                   num_devices=N_CORES)

    TN = cap // TT   # 512-token tiles (mm2)
    TH = cap // HT   # 256-token half tiles (mm1 / x DMA)
    DC = D // P      # d chunks (8)
    HC = H // P      # hh chunks (16)
    JC = H // TT     # wu column blocks of 512
    RR = TT // P     # 128-col sub-blocks per wu block (4)
    HH = HC // 2     # wd halves

    # Host-retiled DRAM layouts (see kernel() for the exact construction):
    #  xt[p, (th, c, tt)]: x token-half th, d-chunk c, token tt  (bf16)
    #  wu[p, ...]: j0 in four 128-col segments (c-major), then j=1..JC-1
    #              512-col blocks (c-major)
    #  wd[p, (g, hh', dd)]: two halves of HC/2 chunks, D cols each
    xt = nc.dram_tensor("xt", [P, TH * DC * HT], bf16, kind="ExternalInput").ap()
    wu = nc.dram_tensor("wu", [P, DC * H], bf16, kind="ExternalInput").ap()
    wd = nc.dram_tensor("wd", [P, HC * D], bf16, kind="ExternalInput").ap()
    ot = nc.dram_tensor("ot", [D, cap], bf16, kind="ExternalOutput").ap()

    with tile.TileContext(nc) as tc:
        with tc.tile_pool(name="sb", bufs=1) as sb, \
             tc.tile_pool(name="ps", bufs=8, space="PSUM") as psp:
          no_dma = "dma" in ablate      # skip input DMA loads
          no_mm1 = "mm1" in ablate      # skip first matmul + relu^2
          no_mm2 = "mm2" in ablate      # skip second matmul
          no_out = "out" in ablate      # skip psum copy + output DMA
          no_warm = "warm" in ablate    # skip PE warmup matmuls

          for _rep in range(repeat):
            # PE warmup: ~3.4us of dummy matmuls on a zeroed tile so the HAM
            # clock gate opens while the input DMAs stream; the real stream
            # then runs warm (2.4GHz) from its first group.
            if not no_warm:
                wrm = sb.tile([P, TT], bf16, tag="warm", name="warm")
                nc.vector.memset(wrm[:], 0)
                wps = psp.tile([P, TT], f32, tag="ps", name="warm_ps")
                for i in range(8):
                    nc.tensor.matmul(wps[:], wrm[:, 0:P], wrm[:],
                                     start=(i == 0), stop=(i == 7))

            # Input DMAs: sync row carries x (ramp-critical for mm1), scalar
            # row carries weights; each HWDGE queue row gets its own share of
            # HBM bandwidth (SDMA engines round-robin rows per packet), and
            # within a row transfers complete in issue order = consumption
            # order below.
            xt_h = {}
            for th in range(TH):
                a = sb.tile([P, DC * HT], bf16, tag=f"xt{th}", name=f"xt{th}")
                if not no_dma:
                    nc.sync.dma_start(a[:], xt[:, th*DC*HT:(th+1)*DC*HT])
                xt_h[th] = a

            wu_seg = {}
            for s in range(RR):
                b = sb.tile([P, DC * P], bf16, tag=f"wu0_{s}", name=f"wu0_{s}")
                if not no_dma:
                    nc.scalar.dma_start(b[:], wu[:, s*DC*P:(s+1)*DC*P])
                wu_seg[s] = b
            wu_j = {}
            for j in range(1, JC):
                off = DC * TT + (j - 1) * DC * TT
                b = sb.tile([P, DC * TT], bf16, tag=f"wu{j}", name=f"wu{j}")
                if not no_dma:
                    nc.scalar.dma_start(b[:], wu[:, off:off + DC*TT])
                wu_j[j] = b
            wd_g = {}
            for g in range(2):
                w = sb.tile([P, HH * D], bf16, tag=f"wd{g}", name=f"wd{g}")
                if not no_dma:
                    nc.scalar.dma_start(w[:], wd[:, g*HH*D:(g+1)*HH*D])
                wd_g[g] = w

            def wu_slice(d, j, rr):
                if j == 0:
                    return wu_seg[rr][:, d*P:(d+1)*P]
                return wu_j[j][:, d*TT + rr*P : d*TT + (rr+1)*P]

            def wd_slice(hh, dd):
                g, h2 = divmod(hh, HH)
                return wd_g[g][:, h2*D + dd*P : h2*D + (dd+1)*P]

            # hT tiles are full 512-token width; mm1 writes them in two
            # 256-token relu halves, mm2 reads them 512 wide.
            hT = {}
            for t in range(TN):
                for hh in range(HC):
                    hT[(hh, t)] = sb.tile([P, TT], bf16, tag=f"h{hh}_{t}",
                                          name=f"h{hh}_{t}")

            # mm1 + fused relu^2, N=256 groups, j-outer: each wu block serves
            # TH*RR psum groups before the next block's DMA is needed, so the
            # PE stream has no DMA-wait gaps after the initial ramp.
            for j in range(JC):
                for th in range(TH):
                    t, half = divmod(th, 2)
                    for rr in range(RR):
                        hh = j * RR + rr
                        if no_mm1:
                            continue
                        ps = psp.tile([P, TT], f32, tag="ps",
                                      name=f"ps1_{th}_{hh}")
                        pv = ps[:, 0:HT]
                        for d in range(DC):
                            nc.tensor.matmul(
                                pv, wu_slice(d, j, rr),
                                xt_h[th][:, d*HT:(d+1)*HT],
                                start=(d == 0), stop=(d == DC - 1))
                        # relu then square on DVE; bf16(relu(x)) == relu(bf16(x))
                        # matches the reference's cast-then-relu, and the bf16
                        # square runs in the DVE 4x SBUF mode.
                        r = sb.tile([P, HT], bf16, tag="relu_tmp", bufs=4,
                                    name=f"r{hh}_{th}")
                        nc.vector.tensor_scalar_max(r[:], pv, 0.0)
                        nc.vector.tensor_tensor(
                            hT[(hh, t)][:, half*HT:(half+1)*HT], r[:], r[:],
                            mybir.AluOpType.mult)

            # mm2: oT[dd*128.., t*512..] = w_downT^T @ hT. The very last
            # group is split into two N=256 halves so the first half's
            # copy + output DMA (and part of its HBM write-completion
            # latency) overlap the second half's matmuls instead of
            # serializing after the final matmul.
            for t in range(TN):
                for dd in range(DC):
                    if no_mm2:
                        continue
                    last = (t == TN - 1 and dd == DC - 1)
                    nh = 2 if last else 1
                    w2 = TT // nh
                    for half in range(nh):
                        ps = psp.tile([P, w2], f32, tag="ps",
                                      name=f"ps2_{t}_{dd}_{half}")
                        for hh in range(HC):
                            nc.tensor.matmul(
                                ps[:], wd_slice(hh, dd),
                                hT[(hh, t)][:, half*w2:(half+1)*w2],
                                start=(hh == 0), stop=(hh == HC - 1))
                        if no_out:
                            continue
                        o = sb.tile([P, w2], bf16, tag=f"o{dd}_{t}_{half}",
                                    name=f"o{dd}_{t}_{half}")
                        nc.vector.tensor_copy(o[:], ps[:])
                        lo = t * TT + half * w2
                        nc.sync.dma_start(ot[dd*P:(dd+1)*P, lo:lo+w2], o[:])

    nc.compile()
    return nc


def _prep_x(xb, s, c, cap, D):
    """Retile one expert's token slice to the kernel's xt layout:
    X[p, (th, cc, tt)] = x[s + th*HT + tt, cc*P + p], zero-padded to cap."""
    TH, DC = cap // HT, D // P
    A = np.zeros((cap, D), BF16)
    if c:
        A[:c] = xb[s:s + c]
    return np.ascontiguousarray(
        A.reshape(TH, HT, DC, P).transpose(3, 0, 2, 1).reshape(P, -1))


def _prep_wu(w_up_e, D, H):
    """[H, D] fp32 -> [P, DC*H] bf16 in the kernel's wu layout: j0 block in
    four 128-col segments (c-major), then 512-col blocks j=1.. (c-major)."""
    DC = D // P
    B = w_up_e.astype(BF16).T.reshape(DC, P, H).transpose(1, 0, 2)  # [P,c,h]
    segs = [B[:, :, s*P:(s+1)*P] for s in range(TT // P)]
    segs += [B[:, :, j*TT:(j+1)*TT] for j in range(1, H // TT)]
    return np.ascontiguousarray(
        np.concatenate([sg.reshape(P, -1) for sg in segs], axis=1))


def _prep_wd(w_down_e, D, H):
    """[D, H] fp32 -> [P, HC*D] bf16: two halves of HC/2 hh-chunks each."""
    HC = H // P
    C = w_down_e.astype(BF16).T.reshape(HC, P, D).transpose(1, 0, 2)
    return np.ascontiguousarray(
        np.concatenate([C[:, :HC//2].reshape(P, -1),
                        C[:, HC//2:].reshape(P, -1)], axis=1))


def _install_ntff_hook():
    """Provide antenv.axon_hooks (missing in some containers) so that
    run_bass_kernel_spmd(trace=True) can capture NTFF profiles via the
    libaxon_pjrt sidechannel. Returns True when tracing is possible."""
    import contextlib
    import ctypes
    import sys
    import types
    try:
        from antenv.axon_hooks import get_axon_ntff_profile_hook  # noqa: F401
        return True
    except ImportError:
        pass
    so_path = "/opt/axon/libaxon_pjrt.so"
    if not os.path.exists(so_path):
        return False
    lib = ctypes.CDLL(so_path)
    if not hasattr(lib, "axon_start_nrt_profile"):
        return False
    lib.axon_start_nrt_profile.argtypes = [ctypes.POINTER(ctypes.c_int64),
                                           ctypes.c_size_t]
    lib.axon_start_nrt_profile.restype = ctypes.c_int64
    lib.axon_stop_nrt_profile.argtypes = [ctypes.c_char_p]
    lib.axon_stop_nrt_profile.restype = ctypes.c_int64

    @contextlib.contextmanager
    def _hook(output_dir, device_ids):
        import jax
        jax.devices()
        if device_ids:
            ids = (ctypes.c_int64 * len(device_ids))(*device_ids)
            rc = lib.axon_start_nrt_profile(ids, len(device_ids))
        else:
            rc = lib.axon_start_nrt_profile(None, 0)
        if rc != 0:
            raise RuntimeError(f"axon_start_nrt_profile rc={rc}")
        try:
            yield
        finally:
            n = lib.axon_stop_nrt_profile(str(output_dir).encode())
            print(f"ntff profile: {n} file(s) in {output_dir}", file=sys.stderr)

    mod = types.ModuleType("antenv.axon_hooks")
    mod.get_axon_ntff_profile_hook = lambda: _hook
    mod.set_axon_ntff_profile_hook = lambda h: None
    sys.modules["antenv.axon_hooks"] = mod
    return True


class _Runner:
    """Jit the bass_exec custom call once per (D, H, cap) so repeat kernel()
    calls skip retracing/recompiling (run_bass_kernel_spmd re-jits per call)."""

    def __init__(self, nc):
        import jax
        import concourse.mybir as mybir
        from jax.sharding import Mesh, NamedSharding, PartitionSpec
        try:
            from jax.experimental.shard_map import shard_map
        except ImportError:
            from jax import shard_map
        from concourse.bass2jax import (
            _bass_exec_p, install_neuronx_cc_hook, partition_id_tensor)

        install_neuronx_cc_hook()
        self.jax = jax
        pname = nc.partition_id_tensor.name if nc.partition_id_tensor else None
        in_names, out_names, out_avals, self.zero_shapes = [], [], [], []
        for alloc in nc.m.functions[0].allocations:
            if not isinstance(alloc, mybir.MemoryLocationSet):
                continue
            name = alloc.memorylocations[0].name
            if alloc.kind == "ExternalInput":
                if name != pname:
                    in_names.append(name)
            elif alloc.kind == "ExternalOutput":
                out_names.append(name)
                shape = tuple(alloc.tensor_shape)
                dtype = mybir.dt.np(alloc.dtype)
                out_avals.append(jax.core.ShapedArray(shape, dtype))
                self.zero_shapes.append((shape, dtype))
        self.in_names, self.out_names, self.out_avals = in_names, out_names, out_avals
        n_params = len(in_names)
        all_names = tuple(in_names + out_names)
        if pname is not None:
            all_names = all_names + (pname,)

        def _body(*args):
            operands = list(args)
            if pname is not None:
                operands.append(partition_id_tensor())
            return tuple(_bass_exec_p.bind(
                *operands, out_avals=tuple(out_avals), in_names=all_names,
                out_names=tuple(out_names), lowering_input_output_aliases=(),
                sim_require_finite=True, sim_require_nnan=True, nc=nc))

        devices = jax.devices()[:N_CORES]
        mesh = Mesh(np.asarray(devices), ("core",))
        spec = PartitionSpec("core")
        self.sharding = NamedSharding(mesh, spec)
        self.fn = jax.jit(shard_map(
            _body, mesh=mesh,
            in_specs=(spec,) * (n_params + len(out_names)),
            out_specs=(spec,) * len(out_names), check_rep=False))

    _zeros_dev = None

    def run(self, in_maps, dev_args=None, concat_args=None):
        """dev_args: optional {name: device_array} of pre-uploaded inputs
        (weights reused across calls). concat_args: optional {name: ndarray}
        already in concatenated (N_CORES*dim0, ...) layout - skips the
        per-core concat copy."""
        jax = self.jax
        dev_args = dev_args or {}
        concat_args = concat_args or {}
        args = []
        for i, n in enumerate(self.in_names):
            if n in dev_args:
                args.append(dev_args[n])
            else:
                a = concat_args.get(n)
                if a is None:
                    a = np.concatenate([np.asarray(m[n]) for m in in_maps],
                                       axis=0)
                args.append(jax.device_put(a, self.sharding))
        # output-placeholder zeros are constant and non-donated: upload once
        if self._zeros_dev is None:
            self._zeros_dev = [
                jax.device_put(np.zeros((N_CORES * s[0], *s[1:]), dt),
                               self.sharding) for s, dt in self.zero_shapes]
        args += self._zeros_dev
        outs = jax.block_until_ready(self.fn(*args))
        return [
            {name: np.asarray(outs[i]).reshape(N_CORES, *self.out_avals[i].shape)[c]
             for i, name in enumerate(self.out_names)}
            for c in range(N_CORES)
        ]

    def put_weights(self, in_maps, names=("wu", "wd")):
        """Upload the per-core weight tensors once; returns {name: dev_array}."""
        jax = self.jax
        out = {}
        for n in names:
            a = np.concatenate([np.asarray(m[n]) for m in in_maps], axis=0)
            out[n] = jax.device_put(a, self.sharding)
        jax.block_until_ready(list(out.values()))
        return out


CAP_MAX = 2048   # per-launch token capacity bound (SBUF: hT tiles scale with cap)


def kernel(x, w_up, w_down, num_tokens_per_expert):
    global LAST_RESULT

    x = np.asarray(x)
    w_up = np.asarray(w_up)
    w_down = np.asarray(w_down)
    counts = np.asarray(num_tokens_per_expert).astype(np.int64)

    T, D = x.shape
    E, H, _ = w_up.shape
    assert E == N_CORES
    ends = np.cumsum(counts)
    starts = ends - counts
    cap = max(TT, int(-(-int(counts.max()) // TT) * TT))
    # Heavily skewed distributions would not fit in SBUF in one pass:
    # process the token range in CAP_MAX chunks per expert.
    cap = min(cap, CAP_MAX)

    key = (D, H, cap)
    if key not in _cache:
        nc = _build(D, H, cap)
        _cache[key] = (nc, _Runner(nc))
    nc, runner = _cache[key]

    xb = x.astype(BF16)
    # Weights are usually identical across calls: cache the retiled bf16
    # host copies AND the device-resident buffers. Fast path: the cache holds
    # references to the exact arrays last seen, so an identity match proves
    # content equality (the address cannot be recycled while referenced);
    # otherwise fall back to a content digest (a changed array re-uploads).
    ident = _wcache.get("ident")
    if ident is not None and ident[0] is w_up and ident[1] is w_down \
            and ident[2] == (D, H, cap):
        wkey = ident[3]
    else:
        import hashlib
        dig = hashlib.blake2b(digest_size=16)
        dig.update(np.ascontiguousarray(w_up).data)
        dig.update(np.ascontiguousarray(w_down).data)
        wkey = (dig.hexdigest(), D, H, cap)
    if wkey not in _wcache:
        for k in list(_wcache):   # hold at most one weight set
            if k != "ident":
                del _wcache[k]
        wub = [_prep_wu(w_up[e], D, H) for e in range(E)]
        wdb = [_prep_wd(w_down[e], D, H) for e in range(E)]
        wmaps = [{"wu": wub[e], "wd": wdb[e]} for e in range(E)]
        _wcache[wkey] = (wub, wdb, runner.put_weights(wmaps))
    _wcache["ident"] = (w_up, w_down, (D, H, cap), wkey)
    wub, wdb, dev_w = _wcache[wkey]

    out = np.zeros((T, D), x.dtype)
    n_launch = max(1, int(-(-int(counts.max()) // cap)))
    xw = cap // HT * (D // P) * HT   # xt row width per core
    for k in range(n_launch):
        s_k = starts + k * cap
        c_k = np.clip(counts - k * cap, 0, cap)
        # token slices built directly in the runner's concatenated layout;
        # in_maps carry zero-copy views for the trace path
        xall = np.zeros((E * P, xw), BF16)
        in_maps = []
        for e in range(E):
            c = int(c_k[e])
            xall[e*P:(e+1)*P] = _prep_x(xb, int(s_k[e]), c, cap, D)
            in_maps.append({"xt": xall[e*P:(e+1)*P],
                            "wu": wub[e], "wd": wdb[e]})

        if os.environ.get("MOE_KERNEL_TRACE") == "1" and _install_ntff_hook():
            from concourse.bass_utils import run_bass_kernel_spmd
            res = run_bass_kernel_spmd(nc, in_maps, list(range(N_CORES)),
                                       trace=True)
            LAST_RESULT = res
            results = res.results
        else:
            results = runner.run(in_maps, dev_args=dev_w,
                                 concat_args={"xt": xall})

        for e in range(E):
            c = int(c_k[e])
            if c:
                out[int(s_k[e]):int(s_k[e]) + c] = \
                    results[e]["ot"][:, :c].T.astype(x.dtype)
    return out


# revision 5
# speedup vs baseline: 1.0079x; 1.0079x over previous
"""Expert-parallel MoE grouped-MLP kernel for 8 TRN2 NeuronCores.

Computes, for tokens t in expert e's contiguous row range (rows of x are
sorted by expert; boundaries come from num_tokens_per_expert):

    out[t] = bf16( relu(bf16(x[t]) @ w_up[e].T)^2 @ w_down[e].T )  -> f32

Strategy (expert parallelism): core e owns expert e. The host does the
dispatch - slices x by expert boundaries, retiles to DMA-friendly layouts,
casts to bf16 - so each core runs two dense back-to-back bf16 matmul
chains entirely on-chip with zero routing logic:

    hT[hh, t] = sum_d w_upT[d, hh] * xT[d, t]        (mm1, PSUM f32)
    hT       <- relu(hT)^2  (cast bf16)               (DVE, fused op)
    oT[dd, t] = sum_hh w_downT[hh, dd] * hT[hh, t]    (mm2, PSUM f32)

Ramp/latency design (the steady-state PE stream is already at the N=512
issue bound of ~213.5ns/MM, so the wins are at the edges):
  - 12 warmup matmuls on a zeroed tile bridge the PE from kernel start to
    the first fed real group, so the HAM clock gate opens (1.2->2.4GHz)
    during the input-DMA ramp and never re-throttles (no PE idle > 3.4us).
  - Ramp-critical loads (x token-tile 0 + w_up block 0, d-interleaved
    pairs, then x tile 1) go on the sync HWDGE row alone, in consumption
    order, so they get the full HBM rate; the bulk weights (w_up j=1..3,
    w_down) issue on the scalar HWDGE row but are release-gated by a
    1-element memset dependency placed after mm1's first relu ops, so
    their transfers cannot steal bandwidth inside the critical window.
  - Host pre-tiles every DRAM operand so each DMA's per-partition lines
    are contiguous (>=2KB) for near-peak HBM efficiency.
  - mm2's last group is split 2x256 so the final copy+output-DMA overlaps
    matmuls instead of serializing after the last one.
"""

import os

import numpy as np
import ml_dtypes

N_CORES = 8
BF16 = ml_dtypes.bfloat16
P = 128          # SBUF/PSUM partitions
TT = 512         # token tile (matmul free dim / one PSUM bank of f32)
N_WARM = 12      # PE warmup matmuls (8 cold ~= 3.4us + 4 warm bridge)

_cache = {}
_wcache = {}  # weight digest -> (host retiled copies, device arrays)
LAST_RESULT = None  # BassKernelResults of the most recent run (for profiling)


def _build(D, H, cap, repeat=1, ablate=()):
    """Compile the per-core Bass program for fixed token capacity `cap`.

    repeat>1 emits the whole body N times into one NEFF (tags shared, so
    iterations serialize through tile reuse) - used only by the timing
    harness to measure per-iteration device time differentially.
    """
    import concourse.mybir as mybir
    import concourse.tile as tile
    from concourse import bacc

    f32 = mybir.dt.float32
    bf16 = mybir.dt.bfloat16

    nc = bacc.Bacc("TRN2", target_bir_lowering=False, debug=False,
                   num_devices=N_CORES)

    TN = cap // TT   # token tiles
    DC = D // P      # d chunks (8)
    HC = H // P      # hh chunks (16)
    JC = H // TT     # wu column blocks of 512
    RR = TT // P     # 128-col sub-blocks per wu block (4)
    HH = HC // 2     # wd halves

    # Host-retiled DRAM layouts (see _prep_* for the exact construction):
    #  xt[p, (t, c, tt)]  wu[p, (j, c, hcol)]  wd[p, (g, hh', dcol)]
    # Every DMA below reads a contiguous [:, a:b] slice.
    xt = nc.dram_tensor("xt", [P, TN * DC * TT], bf16, kind="ExternalInput").ap()
    wu = nc.dram_tensor("wu", [P, DC * H], bf16, kind="ExternalInput").ap()
    wd = nc.dram_tensor("wd", [P, HC * D], bf16, kind="ExternalInput").ap()
    ot = nc.dram_tensor("ot", [D, cap], bf16, kind="ExternalOutput").ap()

    with tile.TileContext(nc) as tc:
        with tc.tile_pool(name="sb", bufs=1) as sb, \
             tc.tile_pool(name="ps", bufs=8, space="PSUM") as psp:
          no_dma = "dma" in ablate      # skip input DMA loads
          no_mm1 = "mm1" in ablate      # skip first matmul + relu^2
          no_mm2 = "mm2" in ablate      # skip second matmul
          no_out = "out" in ablate      # skip psum copy + output DMA
          no_warm = "warm" in ablate    # skip PE warmup matmuls

          for _rep in range(repeat):
            # PE warmup: dummy matmuls on a zeroed tile keep the PE busy from
            # kernel start until the first real group's data lands, so the HAM
            # clock gate opens during the DMA ramp and the real stream runs
            # warm (2.4GHz) from its first matmul.
            if not no_warm:
                wrm = sb.tile([P, TT], bf16, tag="warm", name="warm")
                nc.vector.memset(wrm[:], 0)
                wps = psp.tile([P, TT], f32, tag="ps", name="warm_ps")
                for i in range(N_WARM):
                    nc.tensor.matmul(wps[:], wrm[:, 0:P], wrm[:],
                                     start=(i == 0), stop=(i == N_WARM - 1))

            # Ramp-critical input DMAs on the sync row, d-interleaved in
            # consumption order: (xt t0 d-pair, wu j0 d-pair) x4, then xt t1..
            # Pairing halves the ~0.6us-per-issue sequencer cost.
            xt_t = {}   # (d, t) -> [P, TT] slice
            wu_t = {}   # (d, j) -> [P, TT] slice
            for dp in range(DC // 2):
                d0 = 2 * dp
                a = sb.tile([P, 2, TT], bf16, tag=f"xt0_{dp}", name=f"xt0_{dp}")
                if not no_dma:
                    nc.sync.dma_start(a[:], xt[:, d0*TT:(d0+2)*TT]
                                      .rearrange("p (c t) -> p c t", c=2))
                xt_t[(d0, 0)] = a[:, 0, :]
                xt_t[(d0 + 1, 0)] = a[:, 1, :]
                b = sb.tile([P, 2, TT], bf16, tag=f"wu0_{dp}", name=f"wu0_{dp}")
                if not no_dma:
                    nc.sync.dma_start(b[:], wu[:, d0*TT:(d0+2)*TT]
                                      .rearrange("p (c t) -> p c t", c=2))
                wu_t[(d0, 0)] = b[:, 0, :]
                wu_t[(d0 + 1, 0)] = b[:, 1, :]
            for t in range(1, TN):
                a = sb.tile([P, DC, TT], bf16, tag=f"xt{t}", name=f"xt{t}")
                if not no_dma:
                    nc.sync.dma_start(a[:], xt[:, t*DC*TT:(t+1)*DC*TT]
                                      .rearrange("p (c t) -> p c t", c=DC))
                for d in range(DC):
                    xt_t[(d, t)] = a[:, d, :]

            # Bulk weights on the scalar row. Their transfers are gated: each
            # destination tile first gets a 1-element memset emitted later in
            # the DVE stream (after mm1's first relu ops); the dma_start is
            # emitted after the memset, so the DMA waits on it (WAW) and the
            # transfer stays out of the ramp-critical window. The DMA then
            # overwrites the memset element with the real weight value.
            gated = []
            wu_j = {}
            for j in range(1, JC):
                b = sb.tile([P, DC * TT], bf16, tag=f"wu{j}", name=f"wu{j}")
                wu_j[j] = b
                gated.append((b, wu[:, j*DC*TT:(j+1)*DC*TT]))
            wd_g = {}
            for g in range(2):
                w = sb.tile([P, HH * D], bf16, tag=f"wd{g}", name=f"wd{g}")
                wd_g[g] = w
                gated.append((w, wd[:, g*HH*D:(g+1)*HH*D]))

            def wu_slice(d, j, rr):
                if j == 0:
                    return wu_t[(d, 0)][:, rr*P:(rr+1)*P]
                return wu_j[j][:, d*TT + rr*P : d*TT + (rr+1)*P]

            def wd_slice(hh, dd):
                g, h2 = divmod(hh, HH)
                return wd_g[g][:, h2*D + dd*P : h2*D + (dd+1)*P]

            hT = {}
            for t in range(TN):
                for hh in range(HC):
                    hT[(hh, t)] = sb.tile([P, TT], bf16, tag=f"h{hh}_{t}",
                                          name=f"h{hh}_{t}")

            # mm1 + fused relu^2: j-outer so each wu block serves TN*RR psum
            # groups before the next block's DMA is needed.
            for j in range(JC):
                for t in range(TN):
                    for rr in range(RR):
                        hh = j * RR + rr
                        if no_mm1:
                            continue
                        ps = psp.tile([P, TT], f32, tag="ps",
                                      name=f"ps1_{t}_{hh}")
                        for d in range(DC):
                            nc.tensor.matmul(
                                ps[:], wu_slice(d, j, rr), xt_t[(d, t)],
                                start=(d == 0), stop=(d == DC - 1))
                        # relu then square on DVE; bf16(relu(x)) == relu(bf16(x))
                        # matches the reference's cast-then-relu, and the bf16
                        # square runs in the DVE 4x SBUF mode.
                        r = sb.tile([P, TT], bf16, tag="relu_tmp", bufs=4,
                                    name=f"r{hh}_{t}")
                        nc.vector.tensor_scalar_max(r[:], ps[:], 0.0)
                        nc.vector.tensor_tensor(hT[(hh, t)][:], r[:], r[:],
                                                mybir.AluOpType.mult)
                        if j == 0 and t == 0 and rr == 1 and gated:
                            # release the gated bulk-weight DMAs: by the time
                            # the DVE reaches these 1-element memsets the
                            # ramp-critical transfers are nearly done, and the
                            # bulk still lands with microseconds of deadline
                            # slack before mm1 j=1 / mm2 consume it.
                            for tl, _src in gated:
                                nc.vector.memset(tl[0:1, 0:1], 0)
                            if not no_dma:
                                for tl, src in gated:
                                    nc.scalar.dma_start(tl[:], src)
                            gated = []

            # mm2: oT[dd*128.., t*512..] = w_downT^T @ hT. The very last
            # group is split into two N=256 halves so the first half's
            # copy + output DMA (and part of its HBM write-completion
            # latency) overlap the second half's matmuls instead of
            # serializing after the final matmul.
            for t in range(TN):
                for dd in range(DC):
                    if no_mm2:
                        continue
                    last = (t == TN - 1 and dd == DC - 1)
                    nh = 2 if last else 1
                    w2 = TT // nh
                    for half in range(nh):
                        ps = psp.tile([P, w2], f32, tag="ps",
                                      name=f"ps2_{t}_{dd}_{half}")
                        for hh in range(HC):
                            nc.tensor.matmul(
                                ps[:], wd_slice(hh, dd),
                                hT[(hh, t)][:, half*w2:(half+1)*w2],
                                start=(hh == 0), stop=(hh == HC - 1))
                        if no_out:
                            continue
                        o = sb.tile([P, w2], bf16, tag=f"o{dd}_{t}_{half}",
                                    name=f"o{dd}_{t}_{half}")
                        nc.vector.tensor_copy(o[:], ps[:])
                        lo = t * TT + half * w2
                        nc.sync.dma_start(ot[dd*P:(dd+1)*P, lo:lo+w2], o[:])

    nc.compile()
    return nc


def _prep_x(xb, s, c, cap, D):
    """Retile one expert's token slice to the kernel's xt layout:
    X[p, (t, cc, tt)] = x[s + t*TT + tt, cc*P + p], zero-padded to cap."""
    TN, DC = cap // TT, D // P
    A = np.zeros((cap, D), BF16)
    if c:
        A[:c] = xb[s:s + c]
    return np.ascontiguousarray(
        A.reshape(TN, TT, DC, P).transpose(3, 0, 2, 1).reshape(P, -1))


def _prep_wu(w_up_e, D, H):
    """[H, D] fp32 -> [P, DC*H] bf16 in the kernel's wu layout:
    W[p, (j, c, hc)] = w_up.T[c*P + p, j*TT + hc]."""
    DC, JC = D // P, H // TT
    B = w_up_e.astype(BF16).T.reshape(DC, P, H).transpose(1, 0, 2)  # [P,c,h]
    return np.ascontiguousarray(
        B.reshape(P, DC, JC, TT).transpose(0, 2, 1, 3).reshape(P, -1))


def _prep_wd(w_down_e, D, H):
    """[D, H] fp32 -> [P, HC*D] bf16: two halves of HC/2 hh-chunks each,
    W[p, (g, hh', dc)] = w_down.T[(g*HC/2 + hh')*P + p, dc]."""
    HC = H // P
    C = w_down_e.astype(BF16).T.reshape(HC, P, D).transpose(1, 0, 2)
    return np.ascontiguousarray(
        np.concatenate([C[:, :HC//2].reshape(P, -1),
                        C[:, HC//2:].reshape(P, -1)], axis=1))


def _install_ntff_hook():
    """Provide antenv.axon_hooks (missing in some containers) so that
    run_bass_kernel_spmd(trace=True) can capture NTFF profiles via the
    libaxon_pjrt sidechannel. Returns True when tracing is possible."""
    import contextlib
    import ctypes
    import sys
    import types
    try:
        from antenv.axon_hooks import get_axon_ntff_profile_hook  # noqa: F401
        return True
    except ImportError:
        pass
    so_path = "/opt/axon/libaxon_pjrt.so"
    if not os.path.exists(so_path):
        return False
    lib = ctypes.CDLL(so_path)
    if not hasattr(lib, "axon_start_nrt_profile"):
        return False
    lib.axon_start_nrt_profile.argtypes = [ctypes.POINTER(ctypes.c_int64),
                                           ctypes.c_size_t]
    lib.axon_start_nrt_profile.restype = ctypes.c_int64
    lib.axon_stop_nrt_profile.argtypes = [ctypes.c_char_p]
    lib.axon_stop_nrt_profile.restype = ctypes.c_int64

    @contextlib.contextmanager
    def _hook(output_dir, device_ids):
        import jax
        jax.devices()
        if device_ids:
            ids = (ctypes.c_int64 * len(device_ids))(*device_ids)
            rc = lib.axon_start_nrt_profile(ids, len(device_ids))
        else:
            rc = lib.axon_start_nrt_profile(None, 0)
        if rc != 0:
            raise RuntimeError(f"axon_start_nrt_profile rc={rc}")
        try:
            yield
        finally:
            n = lib.axon_stop_nrt_profile(str(output_dir).encode())
            print(f"ntff profile: {n} file(s) in {output_dir}", file=sys.stderr)

    mod = types.ModuleType("antenv.axon_hooks")
    mod.get_axon_ntff_profile_hook = lambda: _hook
    mod.set_axon_ntff_profile_hook = lambda h: None
    sys.modules["antenv.axon_hooks"] = mod
    return True


class _Runner:
    """Jit the bass_exec custom call once per (D, H, cap) so repeat kernel()
    calls skip retracing/recompiling (run_bass_kernel_spmd re-jits per call)."""

    def __init__(self, nc):
        import jax
        import concourse.mybir as mybir
        from jax.sharding import Mesh, NamedSharding, PartitionSpec
        try:
            from jax.experimental.shard_map import shard_map
        except ImportError:
            from jax import shard_map
        from concourse.bass2jax import (
            _bass_exec_p, install_neuronx_cc_hook, partition_id_tensor)

        install_neuronx_cc_hook()
        self.jax = jax
        pname = nc.partition_id_tensor.name if nc.partition_id_tensor else None
        in_names, out_names, out_avals, self.zero_shapes = [], [], [], []
        for alloc in nc.m.functions[0].allocations:
            if not isinstance(alloc, mybir.MemoryLocationSet):
                continue
            name = alloc.memorylocations[0].name
            if alloc.kind == "ExternalInput":
                if name != pname:
                    in_names.append(name)
            elif alloc.kind == "ExternalOutput":
                out_names.append(name)
                shape = tuple(alloc.tensor_shape)
                dtype = mybir.dt.np(alloc.dtype)
                out_avals.append(jax.core.ShapedArray(shape, dtype))
                self.zero_shapes.append((shape, dtype))
        self.in_names, self.out_names, self.out_avals = in_names, out_names, out_avals
        n_params = len(in_names)
        all_names = tuple(in_names + out_names)
        if pname is not None:
            all_names = all_names + (pname,)

        def _body(*args):
            operands = list(args)
            if pname is not None:
                operands.append(partition_id_tensor())
            return tuple(_bass_exec_p.bind(
                *operands, out_avals=tuple(out_avals), in_names=all_names,
                out_names=tuple(out_names), lowering_input_output_aliases=(),
                sim_require_finite=True, sim_require_nnan=True, nc=nc))

        devices = jax.devices()[:N_CORES]
        mesh = Mesh(np.asarray(devices), ("core",))
        spec = PartitionSpec("core")
        self.sharding = NamedSharding(mesh, spec)
        self.fn = jax.jit(shard_map(
            _body, mesh=mesh,
            in_specs=(spec,) * (n_params + len(out_names)),
            out_specs=(spec,) * len(out_names), check_rep=False))

    _zeros_dev = None

    def run(self, in_maps, dev_args=None, concat_args=None):
        """dev_args: optional {name: device_array} of pre-uploaded inputs
        (weights reused across calls). concat_args: optional {name: ndarray}
        already in concatenated (N_CORES*dim0, ...) layout - skips the
        per-core concat copy."""
        jax = self.jax
        dev_args = dev_args or {}
        concat_args = concat_args or {}
        args = []
        for i, n in enumerate(self.in_names):
            if n in dev_args:
                args.append(dev_args[n])
            else:
                a = concat_args.get(n)
                if a is None:
                    a = np.concatenate([np.asarray(m[n]) for m in in_maps],
                                       axis=0)
                args.append(jax.device_put(a, self.sharding))
        # output-placeholder zeros are constant and non-donated: upload once
        if self._zeros_dev is None:
            self._zeros_dev = [
                jax.device_put(np.zeros((N_CORES * s[0], *s[1:]), dt),
                               self.sharding) for s, dt in self.zero_shapes]
        args += self._zeros_dev
        outs = jax.block_until_ready(self.fn(*args))
        return [
            {name: np.asarray(outs[i]).reshape(N_CORES, *self.out_avals[i].shape)[c]
             for i, name in enumerate(self.out_names)}
            for c in range(N_CORES)
        ]

    def put_weights(self, in_maps, names=("wu", "wd")):
        """Upload the per-core weight tensors once; returns {name: dev_array}."""
        jax = self.jax
        out = {}
        for n in names:
            a = np.concatenate([np.asarray(m[n]) for m in in_maps], axis=0)
            out[n] = jax.device_put(a, self.sharding)
        jax.block_until_ready(list(out.values()))
        return out


CAP_MAX = 2048   # per-launch token capacity bound (SBUF: hT tiles scale with cap)


def kernel(x, w_up, w_down, num_tokens_per_expert):
    global LAST_RESULT

    x = np.asarray(x)
    w_up = np.asarray(w_up)
    w_down = np.asarray(w_down)
    counts = np.asarray(num_tokens_per_expert).astype(np.int64)

    T, D = x.shape
    E, H, _ = w_up.shape
    assert E == N_CORES
    ends = np.cumsum(counts)
    starts = ends - counts
    cap = max(TT, int(-(-int(counts.max()) // TT) * TT))
    # Heavily skewed distributions would not fit in SBUF in one pass:
    # process the token range in CAP_MAX chunks per expert.
    cap = min(cap, CAP_MAX)

    key = (D, H, cap)
    if key not in _cache:
        nc = _build(D, H, cap)
        _cache[key] = (nc, _Runner(nc))
    nc, runner = _cache[key]

    xb = x.astype(BF16)
    # Weights are usually identical across calls: cache the retiled bf16
    # host copies AND the device-resident buffers. Fast path: the cache holds
    # references to the exact arrays last seen, so an identity match proves
    # content equality (the address cannot be recycled while referenced);
    # otherwise fall back to a content digest (a changed array re-uploads).
    ident = _wcache.get("ident")
    if ident is not None and ident[0] is w_up and ident[1] is w_down \
            and ident[2] == (D, H, cap):
        wkey = ident[3]
    else:
        import hashlib
        dig = hashlib.blake2b(digest_size=16)
        dig.update(np.ascontiguousarray(w_up).data)
        dig.update(np.ascontiguousarray(w_down).data)
        wkey = (dig.hexdigest(), D, H, cap)
    if wkey not in _wcache:
        for k in list(_wcache):   # hold at most one weight set
            if k != "ident":
                del _wcache[k]
        wub = [_prep_wu(w_up[e], D, H) for e in range(E)]
        wdb = [_prep_wd(w_down[e], D, H) for e in range(E)]
        wmaps = [{"wu": wub[e], "wd": wdb[e]} for e in range(E)]
        _wcache[wkey] = (wub, wdb, runner.put_weights(wmaps))
    _wcache["ident"] = (w_up, w_down, (D, H, cap), wkey)
    wub, wdb, dev_w = _wcache[wkey]

    out = np.zeros((T, D), x.dtype)
    n_launch = max(1, int(-(-int(counts.max()) // cap)))
    xw = (cap // TT) * (D // P) * TT   # xt row width per core
    for k in range(n_launch):
        s_k = starts + k * cap
        c_k = np.clip(counts - k * cap, 0, cap)
        # token slices built directly in the runner's concatenated layout;
        # in_maps carry zero-copy views for the trace path
        xall = np.zeros((E * P, xw), BF16)
        in_maps = []
        for e in range(E):
            c = int(c_k[e])
            xall[e*P:(e+1)*P] = _prep_x(xb, int(s_k[e]), c, cap, D)
            in_maps.append({"xt": xall[e*P:(e+1)*P],
                            "wu": wub[e], "wd": wdb[e]})

        if os.environ.get("MOE_KERNEL_TRACE") == "1" and _install_ntff_hook():
            from concourse.bass_utils import run_bass_kernel_spmd
            res = run_bass_kernel_spmd(nc, in_maps, list(range(N_CORES)),
                                       trace=True)
            LAST_RESULT = res
            results = res.results
        else:
            results = runner.run(in_maps, dev_args=dev_w,
                                 concat_args={"xt": xall})

        for e in range(E):
            c = int(c_k[e])
            if c:
                out[int(s_k[e]):int(s_k[e]) + c] = \
                    results[e]["ot"][:, :c].T.astype(x.dtype)
    return out
